# revision 37
# baseline (speedup 1.0000x reference)
"""Trainium2 Bass kernel for DeepSeek-style attention (B=2, S=2048, H=2048,
NH=16, NKV=4, HD=128, repeat_interleave GQA quirk, RoPE, causal mask).

Sharding: 8 cores = 2 (batch) x 4 (kv-head group).  Each core computes
q/k/v projections for its kv group (4 q heads share 1 kv head), RoPE,
attention, and a partial o_proj against its 512-column slice of Wo.
The 4 partial o_proj outputs per batch are summed on the host.

All layouts are prepared host-side:
  xT    [H, S]        x transposed (contraction dim major), bf16
  wqT   [H, 512]      Wq slice transposed, bf16
  wkvT  [H, 256]      Wk|Wv slices transposed and concatenated, bf16
  woT   [512, H]      Wo slice transposed (d-major), bf16
  cs    [128, 2S]     rope cos | sin(sign-folded, pre-rotated by 64), bf16
  maskb [128, nblk, 128]  unique "mixed" mask blocks, transposed, x sqrt(HD)

Device algorithm highlights:
  * scores are computed transposed ([k, q] layout) so the exp'd probs tile
    is directly the stationary operand of the P@V matmul - no transposes.
  * softmax denominator comes free from a ones-column appended to V
    (contraction over k accumulates sum(exp) in psum column 128).
  * no max-subtraction in softmax (scores are O(5); exp is safe in f32,
    and softmax is shift-invariant so results match the reference).
  * mask blocks are classified host-side: all-zero blocks add nothing,
    blocks entirely < -30 are skipped (exp underflows to 0 relative to
    in-row survivors), mixed blocks get a DVE add of the stored block.
  * QK scores for two ki tiles share one 2-bank psum tile, so ONE strided
    exp activate covers both: the scalar engine's 352-cycle/instruction
    overhead is the attention co-bottleneck and this halves it.
  * q projections run quarter-outer and are staged psum->SBUF by the
    (otherwise idle) scalar engine right as each quarter finishes: psum
    banks free in ~0.6us, and the rope chains (DVE muls + PE rot64 perm
    + DVE add) run from SBUF, paced one quarter per projection group.
    All rope, including the last head's, completes by early Q0.
  * DMA runs on one trigger queue in strict consumption order; per-chunk
    {wkv, x} pairs keep the sync queue's ~0.6us/trigger cost under the
    x-transfer time so the k/v projections never wait on triggers.
"""

import math
from contextlib import ExitStack

import ml_dtypes
import numpy as np

import concourse.bass as bass
import concourse.mybir as mybir
import concourse.tile as tile
from concourse import bacc
from concourse.bass_utils import run_bass_kernel_spmd
from concourse.masks import make_identity

B, S, H = 2, 2048, 2048
NH, NKV, HD = 16, 4, 128
P = 128
NB = S // P          # 16 s blocks
HC = H // P          # 16 h chunks
HPG = NH // NKV      # 4 q heads per core
QCH = 512            # q chunk width
NQC = S // QCH       # 4 q chunks
SCALE = 1.0 / math.sqrt(HD)
SQRT_HD = math.sqrt(HD)
F32 = mybir.dt.float32
BF16 = mybir.dt.bfloat16
N_CORES = 8


def _classify_mask(mask):
    """Per 128x128 block: 'zero' (no-op), 'skip' (fully masked), or an index
    into the list of unique transposed/pre-scaled mask blocks."""
    kinds = [[None] * NB for _ in range(NB)]
    uniq, blocks = {}, []
    for qi in range(NB):
        for ki in range(NB):
            sub = mask[qi * P:(qi + 1) * P, ki * P:(ki + 1) * P]
            if not sub.any():
                kinds[qi][ki] = "zero"
            elif sub.max() < -30.0:
                kinds[qi][ki] = "skip"
            else:
                # multiplicative form: exp(qk*s + m) == exp(qk*s) * exp(m),
                # so the mask applies to the exp'd probs tile in SBUF -
                # which the otherwise-idle GPSIMD engine can do.
                blkT = np.exp(np.ascontiguousarray(sub.T, dtype=np.float32))
                blkT = blkT.astype(ml_dtypes.bfloat16)
                key = blkT.tobytes()
                if key not in uniq:
                    uniq[key] = len(blocks)
                    blocks.append(blkT)
                kinds[qi][ki] = uniq[key]
    return kinds, blocks


def _build_program(kinds, n_blocks):
    nc = bacc.Bacc()
    xT = nc.declare_dram_parameter("xT", [H, S], BF16, isOutput=False)
    wqT = nc.declare_dram_parameter("wqT", [H, HPG * HD], BF16, isOutput=False)
    wkvT = nc.declare_dram_parameter("wkvT", [H, 2 * HD], BF16, isOutput=False)
    woT = nc.declare_dram_parameter("woT", [HPG * HD, H], BF16, isOutput=False)
    cs = nc.declare_dram_parameter("cs", [HD, 2 * S], BF16, isOutput=False)
    perm = nc.declare_dram_parameter("perm", [P, P], BF16, isOutput=False)
    maskb = None
    if n_blocks:
        maskb = nc.declare_dram_parameter("maskb", [P, n_blocks, P], BF16,
                                          isOutput=False)
    out = nc.declare_dram_parameter("out", [S, H], BF16, isOutput=True)

    with tile.TileContext(nc) as tc, ExitStack() as ctx:
        consts = ctx.enter_context(tc.tile_pool(name="consts", bufs=1))
        xT_sb = consts.tile([P, HC, S], BF16, tag="xT")
        wqT_sb = consts.tile([P, HC, HPG * HD], BF16, tag="wqT")
        wkvT_sb = consts.tile([P, HC, 2 * HD], BF16, tag="wkvT")
        woT_sb = consts.tile([P, HPG, H], BF16, tag="woT")
        cs_sb = consts.tile([P, 2 * S], BF16, tag="cs")

        # ONE DMA queue, strict consumption order.  Early HBM bandwidth is
        # the binding constraint (all 8 cores stream their inputs at once,
        # ~300 GB/s/core aggregate): any second queue running concurrently
        # just slows the x chunks that pace the k/v projections.  In-queue
        # ordering is hardware-enforced, so per-chunk {wkv, x} pairs give
        # the first matmul its operands early, and everything later (wq at
        # ~30us, rope constants, mask/Wo for attention) arrives just ahead
        # of its consumer.
        perm_sb = consts.tile([P, P], BF16, tag="perm")
        for hc in range(HC):
            nc.sync.dma_start(out=wkvT_sb[:, hc, :],
                              in_=wkvT[hc * P:(hc + 1) * P, :])
            if hc == 0:
                # first x chunk in quarters: the first k-proj matmul only
                # reads columns 0:512, so it can start ~1.5us sooner.
                for sq in range(NQC):
                    nc.sync.dma_start(
                        out=xT_sb[:, hc, sq * QCH:(sq + 1) * QCH],
                        in_=xT[hc * P:(hc + 1) * P, sq * QCH:(sq + 1) * QCH])
            else:
                nc.sync.dma_start(out=xT_sb[:, hc, :],
                                  in_=xT[hc * P:(hc + 1) * P, :])
        nc.sync.dma_start(out=wqT_sb[:],
                          in_=wqT.rearrange("(c p) f -> p c f", p=P))
        nc.sync.dma_start(out=perm_sb[:], in_=perm[:])
        nc.sync.dma_start(out=cs_sb[:], in_=cs[:])
        mask_sb = None
        if n_blocks:
            mask_sb = consts.tile([P, n_blocks, P], BF16, tag="maskb")
            nc.sync.dma_start(out=mask_sb[:], in_=maskb[:])
        nc.sync.dma_start(out=woT_sb[:],
                          in_=woT.rearrange("(g p) f -> p g f", p=P))

        # persistent activation buffers
        qrot_sb = consts.tile([P, HPG, S], BF16, tag="qrot")
        krot_sb = consts.tile([P, S], BF16, tag="krot")
        vaug_sb = consts.tile([P, NB, HD + 1], BF16, tag="vaug")
        ident = consts.tile([P, P], BF16, tag="ident")
        make_identity(nc, ident)

        rope_tmp = ctx.enter_context(tc.tile_pool(name="rope_tmp", bufs=3))
        # staging for k + q projection quarters (scalar psum->SBUF copies).
        # Slot HPG holds k; the v transpose staging aliases slot HPG-1:
        # its xbar-DMA readers complete by ~65us (queued behind the input
        # triggers) while h3's q data only lands there at ~85us, so the
        # write-after-read ordering costs nothing.
        stage = consts.tile([P, HPG + 1, NQC, QCH], BF16, tag="stage")
        VSLOT = HPG - 1

        with tc.tile_pool(name="proj_ps", bufs=8, space="PSUM") as proj_ps:
            # k + v projections, h-chunk-major: the PE consumes xT chunks in
            # DMA arrival order (no head-of-line blocking on late chunks).
            # k/v outputs are d-major; 8 accumulators = all 8 psum banks.
            kps = [proj_ps.tile([P, QCH], F32, tag="ps", name=f"kps{i}")
                   for i in range(NQC)]
            vps = [proj_ps.tile([P, QCH], F32, tag="ps", name=f"vps{i}")
                   for i in range(NQC)]
            for hc in range(HC):
                for sq in range(NQC):
                    nc.tensor.matmul(
                        kps[sq][:], wkvT_sb[:, hc, 0:HD],
                        xT_sb[:, hc, sq * QCH:(sq + 1) * QCH],
                        start=(hc == 0), stop=(hc == HC - 1))
                for sq in range(NQC):
                    nc.tensor.matmul(
                        vps[sq][:], wkvT_sb[:, hc, HD:2 * HD],
                        xT_sb[:, hc, sq * QCH:(sq + 1) * QCH],
                        start=(hc == 0), stop=(hc == HC - 1))
            # k and v psum -> bf16 SBUF (vector handles v, scalar k, in
            # parallel); v's transpose to the s-major layout PV needs goes
            # through the DMA xbar transpose engine - zero PE/vector cost,
            # and its ~us latency is irrelevant (vaug is first read in
            # attention, >50us later; the triggers queue behind the input
            # DMA triggers on sync, which is also fine).
            nc.vector.memset(vaug_sb[:, :, HD:HD + 1], 1.0)
            for sq in range(NQC):
                nc.vector.tensor_copy(stage[:, VSLOT, sq, :], vps[sq][:])
                nc.scalar.copy(out=stage[:, HPG, sq, :], in_=kps[sq][:])

            def emit_vtrans(si0, si1):
                for si in range(si0, si1):
                    vt = proj_ps.tile([P, P], BF16, tag="ps")
                    nc.tensor.transpose(
                        vt[:],
                        stage[:, VSLOT, si // 4,
                              (si % 4) * P:(si % 4 + 1) * P],
                        ident[:])
                    nc.vector.tensor_copy(vaug_sb[:, si, 0:HD], vt[:])

            # rope work queue, one quarter per entry; all sources are
            # staged SBUF bf16 so the DVE muls run at the 2x tier with no
            # psum dependencies.
            rope_q = [(stage[:, HPG, sq, :],
                       krot_sb[:, sq * QCH:(sq + 1) * QCH], sq)
                      for sq in range(NQC)]

            def emit_rope(n):
                for _ in range(n):
                    if not rope_q:
                        return
                    src, dst, sq = rope_q.pop(0)
                    csl = slice(sq * QCH, (sq + 1) * QCH)
                    ssl = slice(S + sq * QCH, S + (sq + 1) * QCH)
                    t1 = rope_tmp.tile([P, QCH], BF16, tag="t1")
                    u = rope_tmp.tile([P, QCH], BF16, tag="u")
                    nc.vector.tensor_mul(t1[:], src, cs_sb[:, csl])
                    nc.vector.tensor_mul(u[:], src, cs_sb[:, ssl])
                    us_ps = proj_ps.tile([P, QCH], F32, tag="ps",
                                         name="us_ps")
                    nc.tensor.matmul(us_ps[:], perm_sb[:], u[:],
                                     start=True, stop=True)
                    nc.vector.tensor_add(dst, t1[:], us_ps[:])

            # q projections, quarter-outer: each quarter's psum is staged
            # to SBUF by scalar right as it stops, so its bank frees ~0.6us
            # later and the next head never waits.  Rope chains pop at a
            # steady ONE-GROUP lag: popping a quarter in its own group
            # makes the boundary perm wait out the stage copy + muls
            # (~1.7us PE stall per group), while a deeper lag leaves a
            # chain backlog that head-of-line-blocks Q0's vector work.
            # Early double-pops convert the initial k backlog to lag 1;
            # exactly one chain (the last head's last quarter) drains
            # into Q0.
            pops = {(0, 0): 1, (0, 1): 2, (0, 2): 2, (0, 3): 2}
            for h in range(HPG):
                for sq in range(NQC):
                    qps = proj_ps.tile([P, QCH], F32, tag="ps", name="qps")
                    for hc in range(HC):
                        nc.tensor.matmul(
                            qps[:], wqT_sb[:, hc, h * HD:(h + 1) * HD],
                            xT_sb[:, hc, sq * QCH:(sq + 1) * QCH],
                            start=(hc == 0), stop=(hc == HC - 1))
                    nc.scalar.copy(out=stage[:, h, sq, :], in_=qps[:])
                    if h == 0:
                        # v transposes AFTER the group: the group's psum
                        # bank then comes from the fast scalar kstage
                        # frees, not the slower vector vstage queue
                        emit_vtrans(sq * 4, sq * 4 + 4)
                    rope_q.append((stage[:, h, sq, :],
                                   qrot_sb[:, h, sq * QCH:(sq + 1) * QCH],
                                   sq))
                    emit_rope(pops.get((h, sq), 1))
            # one chain (h3/sq3) deliberately remains: it produces qrot for
            # the LAST attention chunk's last head, so it drains into early
            # Q0 (via o_ps for its perm) instead of stalling the PE here.

        # attention pools (reuse banks freed by proj_ps).  qk tiles are
        # 2-bank pairs ([P, 2, QCH] f32): two ki's scores share one tile so
        # ONE strided exp activate covers both.  2 pair tiles = 4 banks of
        # QK lookahead.  o_ps has 1 bank: mid-attention filler pieces space
        # out enough to hide its copy; the back-to-back tail pieces
        # ping-pong through the idle qk tile halves instead.
        qk_ps = ctx.enter_context(tc.tile_pool(name="qk_ps", bufs=2, space="PSUM"))
        pv_ps = ctx.enter_context(tc.tile_pool(name="pv_ps", bufs=2, space="PSUM"))
        probs_pool = ctx.enter_context(tc.tile_pool(name="probs", bufs=11))
        attnT_pool = ctx.enter_context(tc.tile_pool(name="attnT", bufs=2))
        small = ctx.enter_context(tc.tile_pool(name="small", bufs=8))
        outsb_pool = ctx.enter_context(tc.tile_pool(name="outsb", bufs=2))
        tp_ps = ctx.enter_context(tc.tile_pool(name="tp_ps", bufs=1, space="PSUM"))
        o_ps = ctx.enter_context(tc.tile_pool(name="o_ps", bufs=1, space="PSUM"))

        def oproj_start(Q, attnT, l, copy_eng=None):
            # partial o_proj for row-block l of chunk Q, split into per-oc
            # pieces the caller can interleave as PE filler; one batched
            # output DMA per row-block.  The tail blocks (copy_eng set)
            # run back-to-back after the last QK pair, so they ping-pong
            # their psum through the idle qk pair-tile halves (bank-
            # aligned [P, 512] f32 slices) to pipeline matmuls past the
            # psum->sbuf copies; mid-attention filler pieces use o_ps.
            si = Q * 4 + l
            osb = outsb_pool.tile([P, QCH * 4], BF16, tag="osb", name="osb")

            def emit(ocs):
                for oc in ocs:
                    if copy_eng is not None and oc % 2 == 1:
                        scq = qk_ps.tile([P, 2, QCH], F32, tag="sc",
                                         name="scq")
                        po = scq[:, l % 2, :]
                    else:
                        pot = o_ps.tile([P, QCH], F32, tag="po")
                        po = pot[:]
                    for hh in range(HPG):
                        nc.tensor.matmul(
                            po, attnT[:, hh, l * P:(l + 1) * P],
                            woT_sb[:, hh, oc * QCH:(oc + 1) * QCH],
                            start=(hh == 0), stop=(hh == HPG - 1))
                    if copy_eng is None:
                        nc.vector.tensor_copy(
                            osb[:, oc * QCH:(oc + 1) * QCH], po)
                    else:
                        # tail path: alternate scalar/vector copies so the
                        # last blocks' psum drains on two engines at once,
                        # + per-oc DMA so the writeback streams while later
                        # ocs still compute.
                        if oc % 2 == 0:
                            nc.scalar.copy(out=osb[:, oc * QCH:(oc + 1) * QCH],
                                           in_=po)
                        else:
                            nc.vector.tensor_copy(
                                osb[:, oc * QCH:(oc + 1) * QCH], po)
                        nc.sync.dma_start(
                            out=out[si * P:(si + 1) * P,
                                    oc * QCH:(oc + 1) * QCH],
                            in_=osb[:, oc * QCH:(oc + 1) * QCH])

            def finish():
                if copy_eng is None:
                    nc.sync.dma_start(out=out[si * P:(si + 1) * P, :],
                                      in_=osb[:])

            return emit, finish

        def o_proj_block(Q, attnT, l, copy_eng=None):
            emit, finish = oproj_start(Q, attnT, l, copy_eng)
            emit(range(4))
            finish()

        def emit_deferred_rope():
            # the last projection quarter's rope chain, emitted between
            # Q0-h0's first QK pairs: its perm goes through o_ps (idle in
            # Q0) and its qrot output is only read by the final chunk.
            src, dst, sq = rope_q.pop(0)
            csl = slice(sq * QCH, (sq + 1) * QCH)
            ssl = slice(S + sq * QCH, S + (sq + 1) * QCH)
            t1 = rope_tmp.tile([P, QCH], BF16, tag="t1")
            u = rope_tmp.tile([P, QCH], BF16, tag="u")
            nc.vector.tensor_mul(t1[:], src, cs_sb[:, csl])
            nc.vector.tensor_mul(u[:], src, cs_sb[:, ssl])
            us_ps = o_ps.tile([P, QCH], F32, tag="po", name="us_d")
            nc.tensor.matmul(us_ps[:], perm_sb[:], u[:],
                             start=True, stop=True)
            nc.vector.tensor_add(dst, t1[:], us_ps[:])

        prev = None  # (Q, attnT) pending o_proj, pipelined one chunk behind
        last_pv = []  # final chunk's (attnT, l) pending o_proj, lag 1
        for Q in range(NQC):
            attnT = attnT_pool.tile([P, HPG, QCH], BF16, tag="attnT")
            for h in range(HPG):
                # the previous chunk's o_proj row-block h is the PE filler
                # for this head, split into per-oc pieces spread through
                # the QK pair stream.
                ofill = oproj_start(prev[0], prev[1], h) if prev is not None \
                    else None
                fill_state = [0]

                def fill(upto):
                    if ofill is None:
                        return
                    while fill_state[0] < min(upto, 4):
                        ofill[0]([fill_state[0]])
                        fill_state[0] += 1
                        if fill_state[0] == 4:
                            ofill[1]()

                probs = {}

                def emit_pv(l):
                    # PV for row-block l fires as soon as its diagonal
                    # prob tile is exp'd, interleaving into the QK stream.
                    qi = Q * 4 + l
                    kis = [ki for ki in range(NB)
                           if kinds[qi][ki] != "skip" and ki in probs]
                    if not kis:
                        nc.vector.memset(attnT[:, h, l * P:(l + 1) * P], 0.0)
                    else:
                        pv = pv_ps.tile([P, HD + 1], F32, tag="pv")
                        for j, ki in enumerate(kis):
                            pt_t, pj = probs[ki]
                            nc.tensor.matmul(
                                pv[:], pt_t[:, pj, l * P:(l + 1) * P],
                                vaug_sb[:, ki, :],
                                start=(j == 0), stop=(j == len(kis) - 1))
                        recip = small.tile([P, 1], F32, tag="recip")
                        nc.vector.reciprocal(recip[:], pv[:, HD:HD + 1])
                        attn = small.tile([P, P], BF16, tag="attn")
                        nc.vector.tensor_scalar_mul(
                            out=attn[:], in0=pv[:, 0:HD], scalar1=recip[:])
                        tp = tp_ps.tile([P, P], BF16, tag="tp")
                        nc.tensor.transpose(tp[:], attn[:], ident[:])
                        nc.vector.tensor_copy(attnT[:, h, l * P:(l + 1) * P],
                                              tp[:])
                        if Q == NQC - 1 and h == HPG - 1:
                            # final chunk: its own o_proj interleaves into
                            # the last head's PV stream, one block behind
                            # so the attnT write has landed.
                            last_pv.append(l)
                            if len(last_pv) >= 2:
                                o_proj_block(Q, attnT, last_pv.pop(0),
                                             copy_eng=nc.scalar)

                present = [ki for ki in range(NB)
                           if any(kinds[Q * 4 + l][ki] != "skip"
                                  for l in range(4))]
                pairs = [present[i:i + 2] for i in range(0, len(present), 2)]
                npairs = len(pairs)
                for pi, pair in enumerate(pairs):
                    # union span over the pair so one rectangular activate
                    # covers both halves (the odd extra 128-col strip costs
                    # the PE far less than a second activate's overhead
                    # costs the scalar engine).
                    cols_by_ki = {
                        ki: [l for l in range(4)
                             if kinds[Q * 4 + l][ki] != "skip"]
                        for ki in pair}
                    lo = min(min(c) for c in cols_by_ki.values()) * P
                    hi = (max(max(c) for c in cols_by_ki.values()) + 1) * P
                    sc = qk_ps.tile([P, 2, QCH], F32, tag="sc")
                    for j, ki in enumerate(pair):
                        nc.tensor.matmul(
                            sc[:, j, lo:hi], krot_sb[:, ki * P:(ki + 1) * P],
                            qrot_sb[:, h, Q * QCH + lo:Q * QCH + hi],
                            start=True, stop=True)
                    pt = probs_pool.tile([P, 2, QCH], BF16, tag="pt")
                    nc.scalar.activation(
                        out=pt[:, 0:len(pair), lo:hi],
                        in_=sc[:, 0:len(pair), lo:hi],
                        func=mybir.ActivationFunctionType.Exp, scale=SCALE)
                    # multiplicative mask on the exp'd probs, on GPSIMD:
                    # keeps the mask off the PE (no extra matmul columns)
                    # and off vector (deep in PV-chain work here).  The
                    # masked slice is the diag ki == the LAST accumulation
                    # of its PV chain, so the extra hop's latency hides.
                    for j, ki in enumerate(pair):
                        for l in cols_by_ki[ki]:
                            kind = kinds[Q * 4 + l][ki]
                            if isinstance(kind, int):
                                nc.gpsimd.tensor_mul(
                                    pt[:, j, l * P:(l + 1) * P],
                                    pt[:, j, l * P:(l + 1) * P],
                                    mask_sb[:, kind, :])
                    for j, ki in enumerate(pair):
                        probs[ki] = (pt, j)
                    if npairs >= 4 and pi + 1 in (
                            (npairs + 3) // 4, 2 * ((npairs + 3) // 4),
                            3 * ((npairs + 3) // 4)):
                        fill(fill_state[0] + 1)
                    if Q == 0 and h == 0 and pi == 0 and rope_q:
                        emit_deferred_rope()
                    for ki in pair:
                        if ki >= Q * 4:
                            emit_pv(ki - Q * 4)
                fill(4)
            prev = (Q, attnT)
        for l in last_pv:
            o_proj_block(prev[0], prev[1], l, copy_eng=nc.scalar)

    nc.compile()
    return nc


_PROGRAM_CACHE = {}


def kernel(x, Wq, Wk, Wv, Wo, cos, sin, attention_mask):
    x = np.asarray(x, dtype=np.float32)
    Wq = np.asarray(Wq, dtype=np.float32)
    Wk = np.asarray(Wk, dtype=np.float32)
    Wv = np.asarray(Wv, dtype=np.float32)
    Wo = np.asarray(Wo, dtype=np.float32)
    cos = np.asarray(cos, dtype=np.float32)
    sin = np.asarray(sin, dtype=np.float32)
    mask = np.asarray(attention_mask, dtype=np.float32)[0, 0]

    kinds, blocks = _classify_mask(mask)
    key = (tuple(tuple(str(k) for k in row) for row in kinds), len(blocks))
    if key not in _PROGRAM_CACHE:
        _PROGRAM_CACHE[key] = _build_program(kinds, len(blocks))
    nc = _PROGRAM_CACHE[key]

    bf = ml_dtypes.bfloat16
    cosT = np.ascontiguousarray(cos[0, 0].T).astype(np.float32)
    sinT = np.ascontiguousarray(sin[0, 0].T).astype(np.float32)
    sinT[0:64] *= -1.0                                   # fold rotate_half sign
    sinP = np.concatenate([sinT[64:], sinT[:64]], axis=0)  # pre-rot 64
    cs = np.concatenate([cosT, sinP], axis=1).astype(bf)   # [HD, 2S]
    maskb = np.stack(blocks, axis=1) if blocks else None   # [P, nblk, P] bf16
    dd = np.arange(P)
    permM = (dd[:, None] == (dd[None, :] + 64) % P).astype(bf)

    in_maps = []
    for c in range(N_CORES):
        b, g = c // NKV, c % NKV
        d0, d1 = g * HPG * HD, (g + 1) * HPG * HD
        wkv = np.concatenate(
            [Wk[g * HD:(g + 1) * HD].T, Wv[g * HD:(g + 1) * HD].T], axis=1)
        m = {
            "xT": np.ascontiguousarray(x[b].T).astype(bf),
            "wqT": np.ascontiguousarray(Wq[d0:d1].T).astype(bf),
            "wkvT": np.ascontiguousarray(wkv).astype(bf),
            "woT": np.ascontiguousarray(Wo[:, d0:d1].T).astype(bf),
            "cs": cs,
            "perm": permM,
        }
        if maskb is not None:
            m["maskb"] = maskb
        in_maps.append(m)

    global _last_in_maps
    _last_in_maps = in_maps
    res = run_bass_kernel_spmd(nc, in_maps, list(range(N_CORES))).results
    out = np.zeros((B, S, H), np.float32)
    for c in range(N_CORES):
        out[c // NKV] += np.asarray(res[c]["out"]).astype(np.float32)
    return out


# revision 38
# speedup vs baseline: 1.1749x; 1.1749x over previous
"""Trainium2 Bass kernel for DeepSeek-style attention (B=2, S=2048, H=2048,
NH=16, NKV=4, HD=128, repeat_interleave GQA quirk, RoPE, causal mask).

Sharding: 8 cores = 2 (batch) x 4 (kv-head group).  Each core computes
q/k/v projections for its kv group (4 q heads share 1 kv head), RoPE,
attention, and a partial o_proj against its 512-column slice of Wo.
The 4 partial o_proj outputs per batch are summed on the host.

All layouts are prepared host-side:
  xT    [H, S]        x transposed (contraction dim major), bf16
  wqT   [H, 512]      Wq slice transposed, bf16
  wkvT  [H, 256]      Wk|Wv slices transposed and concatenated, bf16
  woT   [512, H]      Wo slice transposed (d-major), bf16
  cs    [128, 2S]     rope cos | sin(sign-folded, pre-rotated by 64), bf16
  maskb [128, nblk, 128]  unique "mixed" mask blocks, transposed, x sqrt(HD)

Device algorithm highlights:
  * scores are computed transposed ([k, q] layout) so the exp'd probs tile
    is directly the stationary operand of the P@V matmul - no transposes.
  * softmax denominator comes free from a ones-column appended to V
    (contraction over k accumulates sum(exp) in psum column 128).
  * no max-subtraction in softmax (scores are O(5); exp is safe in f32,
    and softmax is shift-invariant so results match the reference).
  * mask blocks are classified host-side: all-zero blocks add nothing,
    blocks entirely < -30 are skipped (exp underflows to 0 relative to
    in-row survivors), mixed blocks get a DVE add of the stored block.
  * QK scores for two ki tiles share one 2-bank psum tile, so ONE strided
    exp activate covers both: the scalar engine's 352-cycle/instruction
    overhead is the attention co-bottleneck and this halves it.
  * q projections run quarter-outer and are staged psum->SBUF by the
    (otherwise idle) scalar engine right as each quarter finishes: psum
    banks free in ~0.6us, and the rope chains (DVE muls + PE rot64 perm
    + DVE add) run from SBUF, paced one quarter per projection group.
    All rope, including the last head's, completes by early Q0.
  * DMA runs on one trigger queue in strict consumption order; per-chunk
    {wkv, x} pairs keep the sync queue's ~0.6us/trigger cost under the
    x-transfer time so the k/v projections never wait on triggers.
"""

import math
from contextlib import ExitStack

import ml_dtypes
import numpy as np

import concourse.bass as bass
import concourse.mybir as mybir
import concourse.tile as tile
from concourse import bacc
from concourse.bass_utils import run_bass_kernel_spmd
from concourse.masks import make_identity

B, S, H = 2, 2048, 2048
NH, NKV, HD = 16, 4, 128
P = 128
NB = S // P          # 16 s blocks
HC = H // P          # 16 h chunks
HPG = NH // NKV      # 4 q heads per core
QCH = 512            # q chunk width
NQC = S // QCH       # 4 q chunks
SCALE = 1.0 / math.sqrt(HD)
SQRT_HD = math.sqrt(HD)
F32 = mybir.dt.float32
BF16 = mybir.dt.bfloat16
N_CORES = 8


def _classify_mask(mask):
    """Per 128x128 block: 'zero' (no-op), 'skip' (fully masked), or an index
    into the list of unique transposed/pre-scaled mask blocks."""
    kinds = [[None] * NB for _ in range(NB)]
    uniq, blocks = {}, []
    for qi in range(NB):
        for ki in range(NB):
            sub = mask[qi * P:(qi + 1) * P, ki * P:(ki + 1) * P]
            if not sub.any():
                kinds[qi][ki] = "zero"
            elif sub.max() < -30.0:
                kinds[qi][ki] = "skip"
            else:
                # multiplicative form: exp(qk*s + m) == exp(qk*s) * exp(m),
                # so the mask applies to the exp'd probs tile in SBUF -
                # which the otherwise-idle GPSIMD engine can do.
                blkT = np.exp(np.ascontiguousarray(sub.T, dtype=np.float32))
                blkT = blkT.astype(ml_dtypes.bfloat16)
                key = blkT.tobytes()
                if key not in uniq:
                    uniq[key] = len(blocks)
                    blocks.append(blkT)
                kinds[qi][ki] = uniq[key]
    return kinds, blocks


def _build_program(kinds, n_blocks):
    nc = bacc.Bacc()
    xT = nc.declare_dram_parameter("xT", [H, S], BF16, isOutput=False)
    wqT = nc.declare_dram_parameter("wqT", [H, HPG * HD], BF16, isOutput=False)
    wkvT = nc.declare_dram_parameter("wkvT", [H, 2 * HD], BF16, isOutput=False)
    woT = nc.declare_dram_parameter("woT", [HPG * HD, H], BF16, isOutput=False)
    cs = nc.declare_dram_parameter("cs", [HD, 2 * S], BF16, isOutput=False)
    perm = nc.declare_dram_parameter("perm", [P, P], BF16, isOutput=False)
    maskb = None
    if n_blocks:
        maskb = nc.declare_dram_parameter("maskb", [P, n_blocks, P], BF16,
                                          isOutput=False)
    out = nc.declare_dram_parameter("out", [S, H], BF16, isOutput=True)

    with tile.TileContext(nc) as tc, ExitStack() as ctx:
        consts = ctx.enter_context(tc.tile_pool(name="consts", bufs=1))
        xT_sb = consts.tile([P, HC, S], BF16, tag="xT")
        wqT_sb = consts.tile([P, HC, HPG * HD], BF16, tag="wqT")
        wkvT_sb = consts.tile([P, HC, 2 * HD], BF16, tag="wkvT")
        woT_sb = consts.tile([P, HPG, H], BF16, tag="woT")
        cs_sb = consts.tile([P, 2 * S], BF16, tag="cs")

        # ONE DMA queue, strict consumption order.  Early HBM bandwidth is
        # the binding constraint (all 8 cores stream their inputs at once,
        # ~300 GB/s/core aggregate): any second queue running concurrently
        # just slows the x chunks that pace the k/v projections.  In-queue
        # ordering is hardware-enforced, so per-chunk {wkv, x} pairs give
        # the first matmul its operands early, and everything later (wq at
        # ~30us, rope constants, mask/Wo for attention) arrives just ahead
        # of its consumer.
        perm_sb = consts.tile([P, P], BF16, tag="perm")
        for hc in range(HC):
            nc.sync.dma_start(out=wkvT_sb[:, hc, :],
                              in_=wkvT[hc * P:(hc + 1) * P, :])
            if hc == 0:
                # first x chunk in quarters: the first k-proj matmul only
                # reads columns 0:512, so it can start ~1.5us sooner.
                for sq in range(NQC):
                    nc.sync.dma_start(
                        out=xT_sb[:, hc, sq * QCH:(sq + 1) * QCH],
                        in_=xT[hc * P:(hc + 1) * P, sq * QCH:(sq + 1) * QCH])
            else:
                nc.sync.dma_start(out=xT_sb[:, hc, :],
                                  in_=xT[hc * P:(hc + 1) * P, :])
        nc.sync.dma_start(out=wqT_sb[:],
                          in_=wqT.rearrange("(c p) f -> p c f", p=P))
        nc.sync.dma_start(out=perm_sb[:], in_=perm[:])
        nc.sync.dma_start(out=cs_sb[:], in_=cs[:])
        mask_sb = None
        if n_blocks:
            mask_sb = consts.tile([P, n_blocks, P], BF16, tag="maskb")
            nc.sync.dma_start(out=mask_sb[:], in_=maskb[:])
        nc.sync.dma_start(out=woT_sb[:],
                          in_=woT.rearrange("(g p) f -> p g f", p=P))

        # persistent activation buffers
        qrot_sb = consts.tile([P, HPG, S], BF16, tag="qrot")
        krot_sb = consts.tile([P, S], BF16, tag="krot")
        vaug_sb = consts.tile([P, NB, HD + 1], BF16, tag="vaug")
        ident = consts.tile([P, P], BF16, tag="ident")
        make_identity(nc, ident)

        rope_tmp = ctx.enter_context(tc.tile_pool(name="rope_tmp", bufs=3))
        # staging for k + q projection quarters (scalar psum->SBUF copies).
        # Slot HPG holds k; the v transpose staging aliases slot HPG-1:
        # its xbar-DMA readers complete by ~65us (queued behind the input
        # triggers) while h3's q data only lands there at ~85us, so the
        # write-after-read ordering costs nothing.
        stage = consts.tile([P, HPG + 1, NQC, QCH], BF16, tag="stage")
        VSLOT = HPG - 1

        with tc.tile_pool(name="proj_ps", bufs=8, space="PSUM") as proj_ps:
            # k + v projections, h-chunk-major: the PE consumes xT chunks in
            # DMA arrival order (no head-of-line blocking on late chunks).
            # k/v outputs are d-major; 8 accumulators = all 8 psum banks.
            kps = [proj_ps.tile([P, QCH], F32, tag="ps", name=f"kps{i}")
                   for i in range(NQC)]
            vps = [proj_ps.tile([P, QCH], F32, tag="ps", name=f"vps{i}")
                   for i in range(NQC)]
            for hc in range(HC):
                for sq in range(NQC):
                    nc.tensor.matmul(
                        kps[sq][:], wkvT_sb[:, hc, 0:HD],
                        xT_sb[:, hc, sq * QCH:(sq + 1) * QCH],
                        start=(hc == 0), stop=(hc == HC - 1))
                for sq in range(NQC):
                    nc.tensor.matmul(
                        vps[sq][:], wkvT_sb[:, hc, HD:2 * HD],
                        xT_sb[:, hc, sq * QCH:(sq + 1) * QCH],
                        start=(hc == 0), stop=(hc == HC - 1))
            # k and v psum -> bf16 SBUF (vector handles v, scalar k, in
            # parallel); v's transpose to the s-major layout PV needs goes
            # through the DMA xbar transpose engine - zero PE/vector cost,
            # and its ~us latency is irrelevant (vaug is first read in
            # attention, >50us later; the triggers queue behind the input
            # DMA triggers on sync, which is also fine).
            nc.vector.memset(vaug_sb[:, :, HD:HD + 1], 1.0)
            for sq in range(NQC):
                nc.vector.tensor_copy(stage[:, VSLOT, sq, :], vps[sq][:])
                nc.scalar.copy(out=stage[:, HPG, sq, :], in_=kps[sq][:])

            def emit_vtrans(si0, si1):
                for si in range(si0, si1):
                    vt = proj_ps.tile([P, P], BF16, tag="ps")
                    nc.tensor.transpose(
                        vt[:],
                        stage[:, VSLOT, si // 4,
                              (si % 4) * P:(si % 4 + 1) * P],
                        ident[:])
                    nc.vector.tensor_copy(vaug_sb[:, si, 0:HD], vt[:])

            # rope work queue, one quarter per entry; all sources are
            # staged SBUF bf16 so the DVE muls run at the 2x tier with no
            # psum dependencies.
            rope_q = [(stage[:, HPG, sq, :],
                       krot_sb[:, sq * QCH:(sq + 1) * QCH], sq)
                      for sq in range(NQC)]

            def emit_rope(n):
                for _ in range(n):
                    if not rope_q:
                        return
                    src, dst, sq = rope_q.pop(0)
                    csl = slice(sq * QCH, (sq + 1) * QCH)
                    ssl = slice(S + sq * QCH, S + (sq + 1) * QCH)
                    t1 = rope_tmp.tile([P, QCH], BF16, tag="t1")
                    u = rope_tmp.tile([P, QCH], BF16, tag="u")
                    nc.vector.tensor_mul(t1[:], src, cs_sb[:, csl])
                    nc.vector.tensor_mul(u[:], src, cs_sb[:, ssl])
                    us_ps = proj_ps.tile([P, QCH], F32, tag="ps",
                                         name="us_ps")
                    nc.tensor.matmul(us_ps[:], perm_sb[:], u[:],
                                     start=True, stop=True)
                    nc.vector.tensor_add(dst, t1[:], us_ps[:])

            # q projections, quarter-outer: each quarter's psum is staged
            # to SBUF by scalar right as it stops, so its bank frees ~0.6us
            # later and the next head never waits.  Rope chains pop at a
            # steady ONE-GROUP lag: popping a quarter in its own group
            # makes the boundary perm wait out the stage copy + muls
            # (~1.7us PE stall per group), while a deeper lag leaves a
            # chain backlog that head-of-line-blocks Q0's vector work.
            # Early double-pops convert the initial k backlog to lag 1;
            # exactly one chain (the last head's last quarter) drains
            # into Q0.
            pops = {(0, 0): 1, (0, 1): 2, (0, 2): 2, (0, 3): 2}
            for h in range(HPG):
                for sq in range(NQC):
                    if h == 0:
                        # v transposes ahead of the group: they absorb the
                        # staging-copy latency at the kv->q boundary
                        emit_vtrans(sq * 4, sq * 4 + 4)
                    qps = proj_ps.tile([P, QCH], F32, tag="ps", name="qps")
                    for hc in range(HC):
                        nc.tensor.matmul(
                            qps[:], wqT_sb[:, hc, h * HD:(h + 1) * HD],
                            xT_sb[:, hc, sq * QCH:(sq + 1) * QCH],
                            start=(hc == 0), stop=(hc == HC - 1))
                    nc.scalar.copy(out=stage[:, h, sq, :], in_=qps[:])
                    rope_q.append((stage[:, h, sq, :],
                                   qrot_sb[:, h, sq * QCH:(sq + 1) * QCH],
                                   sq))
                    emit_rope(pops.get((h, sq), 1))
            # one chain (h3/sq3) deliberately remains: it produces qrot for
            # the LAST attention chunk's last head, so it drains into early
            # Q0 (via o_ps for its perm) instead of stalling the PE here.

        # attention pools (reuse banks freed by proj_ps).  qk tiles are
        # 2-bank pairs ([P, 2, QCH] f32): two ki's scores share one tile so
        # ONE strided exp activate covers both.  2 pair tiles = 4 banks of
        # QK lookahead.  o_ps has 1 bank: mid-attention filler pieces space
        # out enough to hide its copy; the back-to-back tail pieces
        # ping-pong through the idle qk tile halves instead.
        qk_ps = ctx.enter_context(tc.tile_pool(name="qk_ps", bufs=2, space="PSUM"))
        pv_ps = ctx.enter_context(tc.tile_pool(name="pv_ps", bufs=2, space="PSUM"))
        probs_pool = ctx.enter_context(tc.tile_pool(name="probs", bufs=11))
        attnT_pool = ctx.enter_context(tc.tile_pool(name="attnT", bufs=2))
        small = ctx.enter_context(tc.tile_pool(name="small", bufs=8))
        outsb_pool = ctx.enter_context(tc.tile_pool(name="outsb", bufs=2))
        tp_ps = ctx.enter_context(tc.tile_pool(name="tp_ps", bufs=1, space="PSUM"))
        o_ps = ctx.enter_context(tc.tile_pool(name="o_ps", bufs=1, space="PSUM"))

        def oproj_start(Q, attnT, l, copy_eng=None):
            # partial o_proj for row-block l of chunk Q, split into per-oc
            # pieces the caller can interleave as PE filler; one batched
            # output DMA per row-block.  The tail blocks (copy_eng set)
            # run back-to-back after the last QK pair, so they ping-pong
            # their psum through the idle qk pair-tile halves (bank-
            # aligned [P, 512] f32 slices) to pipeline matmuls past the
            # psum->sbuf copies; mid-attention filler pieces use o_ps.
            si = Q * 4 + l
            osb = outsb_pool.tile([P, QCH * 4], BF16, tag="osb", name="osb")

            def emit(ocs):
                for oc in ocs:
                    if copy_eng is not None and oc % 2 == 1:
                        scq = qk_ps.tile([P, 2, QCH], F32, tag="sc",
                                         name="scq")
                        po = scq[:, l % 2, :]
                    else:
                        pot = o_ps.tile([P, QCH], F32, tag="po")
                        po = pot[:]
                    for hh in range(HPG):
                        nc.tensor.matmul(
                            po, attnT[:, hh, l * P:(l + 1) * P],
                            woT_sb[:, hh, oc * QCH:(oc + 1) * QCH],
                            start=(hh == 0), stop=(hh == HPG - 1))
                    if copy_eng is None:
                        nc.vector.tensor_copy(
                            osb[:, oc * QCH:(oc + 1) * QCH], po)
                    else:
                        # tail path: alternate scalar/vector copies so the
                        # last blocks' psum drains on two engines at once,
                        # + per-oc DMA so the writeback streams while later
                        # ocs still compute.
                        if oc % 2 == 0:
                            nc.scalar.copy(out=osb[:, oc * QCH:(oc + 1) * QCH],
                                           in_=po)
                        else:
                            nc.vector.tensor_copy(
                                osb[:, oc * QCH:(oc + 1) * QCH], po)
                        nc.sync.dma_start(
                            out=out[si * P:(si + 1) * P,
                                    oc * QCH:(oc + 1) * QCH],
                            in_=osb[:, oc * QCH:(oc + 1) * QCH])

            def finish():
                if copy_eng is None:
                    nc.sync.dma_start(out=out[si * P:(si + 1) * P, :],
                                      in_=osb[:])

            return emit, finish

        def o_proj_block(Q, attnT, l, copy_eng=None):
            emit, finish = oproj_start(Q, attnT, l, copy_eng)
            emit(range(4))
            finish()

        def emit_deferred_rope():
            # the last projection quarter's rope chain, emitted between
            # Q0-h0's first QK pairs: its perm goes through o_ps (idle in
            # Q0) and its qrot output is only read by the final chunk.
            src, dst, sq = rope_q.pop(0)
            csl = slice(sq * QCH, (sq + 1) * QCH)
            ssl = slice(S + sq * QCH, S + (sq + 1) * QCH)
            t1 = rope_tmp.tile([P, QCH], BF16, tag="t1")
            u = rope_tmp.tile([P, QCH], BF16, tag="u")
            nc.vector.tensor_mul(t1[:], src, cs_sb[:, csl])
            nc.vector.tensor_mul(u[:], src, cs_sb[:, ssl])
            us_ps = o_ps.tile([P, QCH], F32, tag="po", name="us_d")
            nc.tensor.matmul(us_ps[:], perm_sb[:], u[:],
                             start=True, stop=True)
            nc.vector.tensor_add(dst, t1[:], us_ps[:])

        prev = None  # (Q, attnT) pending o_proj, pipelined one chunk behind
        last_pv = []  # final chunk's (attnT, l) pending o_proj, lag 1
        for Q in range(NQC):
            attnT = attnT_pool.tile([P, HPG, QCH], BF16, tag="attnT")
            for h in range(HPG):
                # the previous chunk's o_proj row-block h is the PE filler
                # for this head, split into per-oc pieces spread through
                # the QK pair stream.
                ofill = oproj_start(prev[0], prev[1], h) if prev is not None \
                    else None
                fill_state = [0]

                def fill(upto):
                    if ofill is None:
                        return
                    while fill_state[0] < min(upto, 4):
                        ofill[0]([fill_state[0]])
                        fill_state[0] += 1
                        if fill_state[0] == 4:
                            ofill[1]()

                probs = {}

                def emit_pv(l):
                    # PV for row-block l fires as soon as its diagonal
                    # prob tile is exp'd, interleaving into the QK stream.
                    qi = Q * 4 + l
                    kis = [ki for ki in range(NB)
                           if kinds[qi][ki] != "skip" and ki in probs]
                    if not kis:
                        nc.vector.memset(attnT[:, h, l * P:(l + 1) * P], 0.0)
                    else:
                        pv = pv_ps.tile([P, HD + 1], F32, tag="pv")
                        for j, ki in enumerate(kis):
                            pt_t, pj = probs[ki]
                            nc.tensor.matmul(
                                pv[:], pt_t[:, pj, l * P:(l + 1) * P],
                                vaug_sb[:, ki, :],
                                start=(j == 0), stop=(j == len(kis) - 1))
                        recip = small.tile([P, 1], F32, tag="recip")
                        nc.vector.reciprocal(recip[:], pv[:, HD:HD + 1])
                        attn = small.tile([P, P], BF16, tag="attn")
                        nc.vector.tensor_scalar_mul(
                            out=attn[:], in0=pv[:, 0:HD], scalar1=recip[:])
                        tp = tp_ps.tile([P, P], BF16, tag="tp")
                        nc.tensor.transpose(tp[:], attn[:], ident[:])
                        nc.vector.tensor_copy(attnT[:, h, l * P:(l + 1) * P],
                                              tp[:])
                        if Q == NQC - 1 and h == HPG - 1:
                            # final chunk: its own o_proj interleaves into
                            # the last head's PV stream, one block behind
                            # so the attnT write has landed.
                            last_pv.append(l)
                            if len(last_pv) >= 2:
                                o_proj_block(Q, attnT, last_pv.pop(0),
                                             copy_eng=nc.scalar)

                present = [ki for ki in range(NB)
                           if any(kinds[Q * 4 + l][ki] != "skip"
                                  for l in range(4))]
                pairs = [present[i:i + 2] for i in range(0, len(present), 2)]
                npairs = len(pairs)
                for pi, pair in enumerate(pairs):
                    # union span over the pair so one rectangular activate
                    # covers both halves (the odd extra 128-col strip costs
                    # the PE far less than a second activate's overhead
                    # costs the scalar engine).
                    cols_by_ki = {
                        ki: [l for l in range(4)
                             if kinds[Q * 4 + l][ki] != "skip"]
                        for ki in pair}
                    lo = min(min(c) for c in cols_by_ki.values()) * P
                    hi = (max(max(c) for c in cols_by_ki.values()) + 1) * P
                    sc = qk_ps.tile([P, 2, QCH], F32, tag="sc")
                    for j, ki in enumerate(pair):
                        nc.tensor.matmul(
                            sc[:, j, lo:hi], krot_sb[:, ki * P:(ki + 1) * P],
                            qrot_sb[:, h, Q * QCH + lo:Q * QCH + hi],
                            start=True, stop=True)
                    pt = probs_pool.tile([P, 2, QCH], BF16, tag="pt")
                    nc.scalar.activation(
                        out=pt[:, 0:len(pair), lo:hi],
                        in_=sc[:, 0:len(pair), lo:hi],
                        func=mybir.ActivationFunctionType.Exp, scale=SCALE)
                    # multiplicative mask on the exp'd probs, on GPSIMD:
                    # keeps the mask off the PE (no extra matmul columns)
                    # and off vector (deep in PV-chain work here).  The
                    # masked slice is the diag ki == the LAST accumulation
                    # of its PV chain, so the extra hop's latency hides.
                    for j, ki in enumerate(pair):
                        for l in cols_by_ki[ki]:
                            kind = kinds[Q * 4 + l][ki]
                            if isinstance(kind, int):
                                nc.gpsimd.tensor_mul(
                                    pt[:, j, l * P:(l + 1) * P],
                                    pt[:, j, l * P:(l + 1) * P],
                                    mask_sb[:, kind, :])
                    for j, ki in enumerate(pair):
                        probs[ki] = (pt, j)
                    if npairs >= 4 and pi + 1 in (
                            (npairs + 3) // 4, 2 * ((npairs + 3) // 4),
                            3 * ((npairs + 3) // 4)):
                        fill(fill_state[0] + 1)
                    if Q == 0 and h == 0 and pi == 0 and rope_q:
                        emit_deferred_rope()
                    for ki in pair:
                        if ki >= Q * 4:
                            emit_pv(ki - Q * 4)
                fill(4)
            prev = (Q, attnT)
        for l in last_pv:
            o_proj_block(prev[0], prev[1], l, copy_eng=nc.scalar)

    nc.compile()
    return nc


_PROGRAM_CACHE = {}


def kernel(x, Wq, Wk, Wv, Wo, cos, sin, attention_mask):
    x = np.asarray(x, dtype=np.float32)
    Wq = np.asarray(Wq, dtype=np.float32)
    Wk = np.asarray(Wk, dtype=np.float32)
    Wv = np.asarray(Wv, dtype=np.float32)
    Wo = np.asarray(Wo, dtype=np.float32)
    cos = np.asarray(cos, dtype=np.float32)
    sin = np.asarray(sin, dtype=np.float32)
    mask = np.asarray(attention_mask, dtype=np.float32)[0, 0]

    kinds, blocks = _classify_mask(mask)
    key = (tuple(tuple(str(k) for k in row) for row in kinds), len(blocks))
    if key not in _PROGRAM_CACHE:
        _PROGRAM_CACHE[key] = _build_program(kinds, len(blocks))
    nc = _PROGRAM_CACHE[key]

    bf = ml_dtypes.bfloat16
    cosT = np.ascontiguousarray(cos[0, 0].T).astype(np.float32)
    sinT = np.ascontiguousarray(sin[0, 0].T).astype(np.float32)
    sinT[0:64] *= -1.0                                   # fold rotate_half sign
    sinP = np.concatenate([sinT[64:], sinT[:64]], axis=0)  # pre-rot 64
    cs = np.concatenate([cosT, sinP], axis=1).astype(bf)   # [HD, 2S]
    maskb = np.stack(blocks, axis=1) if blocks else None   # [P, nblk, P] bf16
    dd = np.arange(P)
    permM = (dd[:, None] == (dd[None, :] + 64) % P).astype(bf)

    in_maps = []
    for c in range(N_CORES):
        b, g = c // NKV, c % NKV
        d0, d1 = g * HPG * HD, (g + 1) * HPG * HD
        wkv = np.concatenate(
            [Wk[g * HD:(g + 1) * HD].T, Wv[g * HD:(g + 1) * HD].T], axis=1)
        m = {
            "xT": np.ascontiguousarray(x[b].T).astype(bf),
            "wqT": np.ascontiguousarray(Wq[d0:d1].T).astype(bf),
            "wkvT": np.ascontiguousarray(wkv).astype(bf),
            "woT": np.ascontiguousarray(Wo[:, d0:d1].T).astype(bf),
            "cs": cs,
            "perm": permM,
        }
        if maskb is not None:
            m["maskb"] = maskb
        in_maps.append(m)

    global _last_in_maps
    _last_in_maps = in_maps
    res = run_bass_kernel_spmd(nc, in_maps, list(range(N_CORES))).results
    out = np.zeros((B, S, H), np.float32)
    for c in range(N_CORES):
        out[c // NKV] += np.asarray(res[c]["out"]).astype(np.float32)
    return out


# revision 39
# speedup vs baseline: 1.1756x; 1.0005x over previous
"""Trainium2 Bass kernel for DeepSeek-style attention (B=2, S=2048, H=2048,
NH=16, NKV=4, HD=128, repeat_interleave GQA quirk, RoPE, causal mask).

Sharding: 8 cores = 2 (batch) x 4 (kv-head group).  Each core computes
q/k/v projections for its kv group (4 q heads share 1 kv head), RoPE,
attention, and a partial o_proj against its 512-column slice of Wo.
The 4 partial o_proj outputs per batch are summed on the host.

All layouts are prepared host-side:
  xT    [H, S]        x transposed (contraction dim major), bf16
  wqT   [H, 512]      Wq slice transposed, bf16
  wkvT  [H, 256]      Wk|Wv slices transposed and concatenated, bf16
  woT   [512, H]      Wo slice transposed (d-major), bf16
  cs    [128, 2S]     rope cos | sin(sign-folded, pre-rotated by 64), bf16
  maskb [128, nblk, 128]  unique "mixed" mask blocks, transposed, x sqrt(HD)

Device algorithm highlights:
  * scores are computed transposed ([k, q] layout) so the exp'd probs tile
    is directly the stationary operand of the P@V matmul - no transposes.
  * softmax denominator comes free from a ones-column appended to V
    (contraction over k accumulates sum(exp) in psum column 128).
  * no max-subtraction in softmax (scores are O(5); exp is safe in f32,
    and softmax is shift-invariant so results match the reference).
  * mask blocks are classified host-side: all-zero blocks add nothing,
    blocks entirely < -30 are skipped (exp underflows to 0 relative to
    in-row survivors), mixed blocks get a DVE add of the stored block.
  * QK scores for two ki tiles share one 2-bank psum tile, so ONE strided
    exp activate covers both: the scalar engine's 352-cycle/instruction
    overhead is the attention co-bottleneck and this halves it.
  * q projections run quarter-outer and are staged psum->SBUF by the
    (otherwise idle) scalar engine right as each quarter finishes: psum
    banks free in ~0.6us, and the rope chains (DVE muls + PE rot64 perm
    + DVE add) run from SBUF, paced one quarter per projection group.
    All rope, including the last head's, completes by early Q0.
  * DMA runs on one trigger queue in strict consumption order; per-chunk
    {wkv, x} pairs keep the sync queue's ~0.6us/trigger cost under the
    x-transfer time so the k/v projections never wait on triggers.
"""

import math
from contextlib import ExitStack

import ml_dtypes
import numpy as np

import concourse.bass as bass
import concourse.mybir as mybir
import concourse.tile as tile
from concourse import bacc
from concourse.bass_utils import run_bass_kernel_spmd
from concourse.masks import make_identity

B, S, H = 2, 2048, 2048
NH, NKV, HD = 16, 4, 128
P = 128
NB = S // P          # 16 s blocks
HC = H // P          # 16 h chunks
HPG = NH // NKV      # 4 q heads per core
QCH = 512            # q chunk width
NQC = S // QCH       # 4 q chunks
SCALE = 1.0 / math.sqrt(HD)
SQRT_HD = math.sqrt(HD)
F32 = mybir.dt.float32
BF16 = mybir.dt.bfloat16
N_CORES = 8


def _classify_mask(mask):
    """Per 128x128 block: 'zero' (no-op), 'skip' (fully masked), or an index
    into the list of unique transposed/pre-scaled mask blocks."""
    kinds = [[None] * NB for _ in range(NB)]
    uniq, blocks = {}, []
    for qi in range(NB):
        for ki in range(NB):
            sub = mask[qi * P:(qi + 1) * P, ki * P:(ki + 1) * P]
            if not sub.any():
                kinds[qi][ki] = "zero"
            elif sub.max() < -30.0:
                kinds[qi][ki] = "skip"
            else:
                # multiplicative form: exp(qk*s + m) == exp(qk*s) * exp(m),
                # so the mask applies to the exp'd probs tile in SBUF -
                # which the otherwise-idle GPSIMD engine can do.
                blkT = np.exp(np.ascontiguousarray(sub.T, dtype=np.float32))
                blkT = blkT.astype(ml_dtypes.bfloat16)
                key = blkT.tobytes()
                if key not in uniq:
                    uniq[key] = len(blocks)
                    blocks.append(blkT)
                kinds[qi][ki] = uniq[key]
    return kinds, blocks


def _build_program(kinds, n_blocks):
    nc = bacc.Bacc()
    xT = nc.declare_dram_parameter("xT", [H, S], BF16, isOutput=False)
    wqT = nc.declare_dram_parameter("wqT", [H, HPG * HD], BF16, isOutput=False)
    wkvT = nc.declare_dram_parameter("wkvT", [H, 2 * HD], BF16, isOutput=False)
    woT = nc.declare_dram_parameter("woT", [HPG * HD, H], BF16, isOutput=False)
    cs = nc.declare_dram_parameter("cs", [HD, 2 * S], BF16, isOutput=False)
    perm = nc.declare_dram_parameter("perm", [P, P], BF16, isOutput=False)
    maskb = None
    if n_blocks:
        maskb = nc.declare_dram_parameter("maskb", [P, n_blocks, P], BF16,
                                          isOutput=False)
    out = nc.declare_dram_parameter("out", [S, H], BF16, isOutput=True)

    with tile.TileContext(nc) as tc, ExitStack() as ctx:
        consts = ctx.enter_context(tc.tile_pool(name="consts", bufs=1))
        xT_sb = consts.tile([P, HC, S], BF16, tag="xT")
        wqT_sb = consts.tile([P, HC, HPG * HD], BF16, tag="wqT")
        wkvT_sb = consts.tile([P, HC, 2 * HD], BF16, tag="wkvT")
        woT_sb = consts.tile([P, HPG, H], BF16, tag="woT")
        cs_sb = consts.tile([P, 2 * S], BF16, tag="cs")

        # ONE DMA queue, strict consumption order.  Early HBM bandwidth is
        # the binding constraint (all 8 cores stream their inputs at once,
        # ~300 GB/s/core aggregate): any second queue running concurrently
        # just slows the x chunks that pace the k/v projections.  In-queue
        # ordering is hardware-enforced, so per-chunk {wkv, x} pairs give
        # the first matmul its operands early, and everything later (wq at
        # ~30us, rope constants, mask/Wo for attention) arrives just ahead
        # of its consumer.
        perm_sb = consts.tile([P, P], BF16, tag="perm")
        for hc in range(HC):
            nc.sync.dma_start(out=wkvT_sb[:, hc, :],
                              in_=wkvT[hc * P:(hc + 1) * P, :])
            if hc == 0:
                # first x chunk in quarters: the first k-proj matmul only
                # reads columns 0:512, so it can start ~1.5us sooner.
                for sq in range(NQC):
                    nc.sync.dma_start(
                        out=xT_sb[:, hc, sq * QCH:(sq + 1) * QCH],
                        in_=xT[hc * P:(hc + 1) * P, sq * QCH:(sq + 1) * QCH])
            else:
                nc.sync.dma_start(out=xT_sb[:, hc, :],
                                  in_=xT[hc * P:(hc + 1) * P, :])
        nc.sync.dma_start(out=wqT_sb[:],
                          in_=wqT.rearrange("(c p) f -> p c f", p=P))
        nc.sync.dma_start(out=perm_sb[:], in_=perm[:])
        nc.sync.dma_start(out=cs_sb[:], in_=cs[:])
        mask_sb = None
        if n_blocks:
            mask_sb = consts.tile([P, n_blocks, P], BF16, tag="maskb")
            nc.sync.dma_start(out=mask_sb[:], in_=maskb[:])
        nc.sync.dma_start(out=woT_sb[:],
                          in_=woT.rearrange("(g p) f -> p g f", p=P))

        # persistent activation buffers
        qrot_sb = consts.tile([P, HPG, S], BF16, tag="qrot")
        krot_sb = consts.tile([P, S], BF16, tag="krot")
        vaug_sb = consts.tile([P, NB, HD + 1], BF16, tag="vaug")
        ident = consts.tile([P, P], BF16, tag="ident")
        make_identity(nc, ident)

        rope_tmp = ctx.enter_context(tc.tile_pool(name="rope_tmp", bufs=3))
        # staging for k + q projection quarters (scalar psum->SBUF copies).
        # Slot HPG holds k; the v transpose staging aliases slot HPG-1:
        # its xbar-DMA readers complete by ~65us (queued behind the input
        # triggers) while h3's q data only lands there at ~85us, so the
        # write-after-read ordering costs nothing.
        stage = consts.tile([P, HPG + 1, NQC, QCH], BF16, tag="stage")
        VSLOT = HPG - 1

        with tc.tile_pool(name="proj_ps", bufs=8, space="PSUM") as proj_ps:
            # k + v projections, h-chunk-major: the PE consumes xT chunks in
            # DMA arrival order (no head-of-line blocking on late chunks).
            # k/v outputs are d-major; 8 accumulators = all 8 psum banks.
            kps = [proj_ps.tile([P, QCH], F32, tag="ps", name=f"kps{i}")
                   for i in range(NQC)]
            vps = [proj_ps.tile([P, QCH], F32, tag="ps", name=f"vps{i}")
                   for i in range(NQC)]
            for hc in range(HC):
                for sq in range(NQC):
                    nc.tensor.matmul(
                        kps[sq][:], wkvT_sb[:, hc, 0:HD],
                        xT_sb[:, hc, sq * QCH:(sq + 1) * QCH],
                        start=(hc == 0), stop=(hc == HC - 1))
                for sq in range(NQC):
                    nc.tensor.matmul(
                        vps[sq][:], wkvT_sb[:, hc, HD:2 * HD],
                        xT_sb[:, hc, sq * QCH:(sq + 1) * QCH],
                        start=(hc == 0), stop=(hc == HC - 1))
            # k and v psum -> bf16 SBUF (vector handles v, scalar k, in
            # parallel); v's transpose to the s-major layout PV needs goes
            # through the DMA xbar transpose engine - zero PE/vector cost,
            # and its ~us latency is irrelevant (vaug is first read in
            # attention, >50us later; the triggers queue behind the input
            # DMA triggers on sync, which is also fine).
            nc.vector.memset(vaug_sb[:, :, HD:HD + 1], 1.0)
            for sq in range(NQC):
                nc.vector.tensor_copy(stage[:, VSLOT, sq, :], vps[sq][:])
                nc.scalar.copy(out=stage[:, HPG, sq, :], in_=kps[sq][:])

            def emit_vtrans(si0, si1):
                for si in range(si0, si1):
                    vt = proj_ps.tile([P, P], BF16, tag="ps")
                    nc.tensor.transpose(
                        vt[:],
                        stage[:, VSLOT, si // 4,
                              (si % 4) * P:(si % 4 + 1) * P],
                        ident[:])
                    nc.vector.tensor_copy(vaug_sb[:, si, 0:HD], vt[:])

            # rope work queue, one quarter per entry; all sources are
            # staged SBUF bf16 so the DVE muls run at the 2x tier with no
            # psum dependencies.
            rope_q = [(stage[:, HPG, sq, :],
                       krot_sb[:, sq * QCH:(sq + 1) * QCH], sq)
                      for sq in range(NQC)]

            def emit_rope(n):
                for _ in range(n):
                    if not rope_q:
                        return
                    src, dst, sq = rope_q.pop(0)
                    csl = slice(sq * QCH, (sq + 1) * QCH)
                    ssl = slice(S + sq * QCH, S + (sq + 1) * QCH)
                    t1 = rope_tmp.tile([P, QCH], BF16, tag="t1")
                    u = rope_tmp.tile([P, QCH], BF16, tag="u")
                    nc.vector.tensor_mul(t1[:], src, cs_sb[:, csl])
                    nc.vector.tensor_mul(u[:], src, cs_sb[:, ssl])
                    us_ps = proj_ps.tile([P, QCH], F32, tag="ps",
                                         name="us_ps")
                    nc.tensor.matmul(us_ps[:], perm_sb[:], u[:],
                                     start=True, stop=True)
                    nc.vector.tensor_add(dst, t1[:], us_ps[:])

            # q projections, quarter-outer: each quarter's psum is staged
            # to SBUF by scalar right as it stops, so its bank frees ~0.6us
            # later and the next head never waits.  Rope chains pop at a
            # steady ONE-GROUP lag: popping a quarter in its own group
            # makes the boundary perm wait out the stage copy + muls
            # (~1.7us PE stall per group), while a deeper lag leaves a
            # chain backlog that head-of-line-blocks Q0's vector work.
            # Early double-pops convert the initial k backlog to lag 1;
            # exactly one chain (the last head's last quarter) drains
            # into Q0.
            pops = {(0, 0): 1, (0, 1): 2, (0, 2): 2, (0, 3): 2}
            for h in range(HPG):
                for sq in range(NQC):
                    if h == 0:
                        # v transposes ahead of the group: they absorb the
                        # staging-copy latency at the kv->q boundary
                        emit_vtrans(sq * 4, sq * 4 + 4)
                    qps = proj_ps.tile([P, QCH], F32, tag="ps", name="qps")
                    for hc in range(HC):
                        nc.tensor.matmul(
                            qps[:], wqT_sb[:, hc, h * HD:(h + 1) * HD],
                            xT_sb[:, hc, sq * QCH:(sq + 1) * QCH],
                            start=(hc == 0), stop=(hc == HC - 1))
                    if h == HPG - 1:
                        # last head: stage on vector so scalar is already
                        # free for Q0's first exp activates
                        nc.vector.tensor_copy(stage[:, h, sq, :], qps[:])
                    else:
                        nc.scalar.copy(out=stage[:, h, sq, :], in_=qps[:])
                    rope_q.append((stage[:, h, sq, :],
                                   qrot_sb[:, h, sq * QCH:(sq + 1) * QCH],
                                   sq))
                    emit_rope(pops.get((h, sq), 1))
            # one chain (h3/sq3) deliberately remains: it produces qrot for
            # the LAST attention chunk's last head, so it drains into early
            # Q0 (via o_ps for its perm) instead of stalling the PE here.

        # attention pools (reuse banks freed by proj_ps).  qk tiles are
        # 2-bank pairs ([P, 2, QCH] f32): two ki's scores share one tile so
        # ONE strided exp activate covers both.  2 pair tiles = 4 banks of
        # QK lookahead.  o_ps has 1 bank: mid-attention filler pieces space
        # out enough to hide its copy; the back-to-back tail pieces
        # ping-pong through the idle qk tile halves instead.
        qk_ps = ctx.enter_context(tc.tile_pool(name="qk_ps", bufs=2, space="PSUM"))
        pv_ps = ctx.enter_context(tc.tile_pool(name="pv_ps", bufs=2, space="PSUM"))
        probs_pool = ctx.enter_context(tc.tile_pool(name="probs", bufs=12))
        attnT_pool = ctx.enter_context(tc.tile_pool(name="attnT", bufs=2))
        small = ctx.enter_context(tc.tile_pool(name="small", bufs=8))
        outsb_pool = ctx.enter_context(tc.tile_pool(name="outsb", bufs=2))
        tp_ps = ctx.enter_context(tc.tile_pool(name="tp_ps", bufs=1, space="PSUM"))
        o_ps = ctx.enter_context(tc.tile_pool(name="o_ps", bufs=1, space="PSUM"))

        def oproj_start(Q, attnT, l, copy_eng=None):
            # partial o_proj for row-block l of chunk Q, split into per-oc
            # pieces the caller can interleave as PE filler; one batched
            # output DMA per row-block.  The tail blocks (copy_eng set)
            # run back-to-back after the last QK pair, so they ping-pong
            # their psum through the idle qk pair-tile halves (bank-
            # aligned [P, 512] f32 slices) to pipeline matmuls past the
            # psum->sbuf copies; mid-attention filler pieces use o_ps.
            si = Q * 4 + l
            osb = outsb_pool.tile([P, QCH * 4], BF16, tag="osb", name="osb")

            def emit(ocs):
                for oc in ocs:
                    if copy_eng is not None and oc % 2 == 1:
                        scq = qk_ps.tile([P, 2, QCH], F32, tag="sc",
                                         name="scq")
                        po = scq[:, l % 2, :]
                    else:
                        pot = o_ps.tile([P, QCH], F32, tag="po")
                        po = pot[:]
                    for hh in range(HPG):
                        nc.tensor.matmul(
                            po, attnT[:, hh, l * P:(l + 1) * P],
                            woT_sb[:, hh, oc * QCH:(oc + 1) * QCH],
                            start=(hh == 0), stop=(hh == HPG - 1))
                    if copy_eng is None:
                        nc.vector.tensor_copy(
                            osb[:, oc * QCH:(oc + 1) * QCH], po)
                    else:
                        # tail path: alternate scalar/vector copies so the
                        # last blocks' psum drains on two engines at once,
                        # + per-oc DMA so the writeback streams while later
                        # ocs still compute.
                        if oc % 2 == 0:
                            nc.scalar.copy(out=osb[:, oc * QCH:(oc + 1) * QCH],
                                           in_=po)
                        else:
                            nc.vector.tensor_copy(
                                osb[:, oc * QCH:(oc + 1) * QCH], po)
                        nc.sync.dma_start(
                            out=out[si * P:(si + 1) * P,
                                    oc * QCH:(oc + 1) * QCH],
                            in_=osb[:, oc * QCH:(oc + 1) * QCH])

            def finish():
                if copy_eng is None:
                    nc.sync.dma_start(out=out[si * P:(si + 1) * P, :],
                                      in_=osb[:])

            return emit, finish

        def o_proj_block(Q, attnT, l, copy_eng=None):
            emit, finish = oproj_start(Q, attnT, l, copy_eng)
            emit(range(4))
            finish()

        def emit_deferred_rope():
            # the last projection quarter's rope chain, emitted between
            # Q0-h0's first QK pairs: its perm goes through o_ps (idle in
            # Q0) and its qrot output is only read by the final chunk.
            src, dst, sq = rope_q.pop(0)
            csl = slice(sq * QCH, (sq + 1) * QCH)
            ssl = slice(S + sq * QCH, S + (sq + 1) * QCH)
            t1 = rope_tmp.tile([P, QCH], BF16, tag="t1")
            u = rope_tmp.tile([P, QCH], BF16, tag="u")
            nc.vector.tensor_mul(t1[:], src, cs_sb[:, csl])
            nc.vector.tensor_mul(u[:], src, cs_sb[:, ssl])
            us_ps = o_ps.tile([P, QCH], F32, tag="po", name="us_d")
            nc.tensor.matmul(us_ps[:], perm_sb[:], u[:],
                             start=True, stop=True)
            nc.vector.tensor_add(dst, t1[:], us_ps[:])

        prev = None  # (Q, attnT) pending o_proj, pipelined one chunk behind
        last_pv = []  # final chunk's (attnT, l) pending o_proj, lag 1
        for Q in range(NQC):
            attnT = attnT_pool.tile([P, HPG, QCH], BF16, tag="attnT")
            for h in range(HPG):
                # the previous chunk's o_proj row-block h is the PE filler
                # for this head, split into per-oc pieces spread through
                # the QK pair stream.
                ofill = oproj_start(prev[0], prev[1], h) if prev is not None \
                    else None
                fill_state = [0]

                def fill(upto):
                    if ofill is None:
                        return
                    while fill_state[0] < min(upto, 4):
                        ofill[0]([fill_state[0]])
                        fill_state[0] += 1
                        if fill_state[0] == 4:
                            ofill[1]()

                probs = {}

                def emit_pv(l):
                    # PV for row-block l fires as soon as its diagonal
                    # prob tile is exp'd, interleaving into the QK stream.
                    qi = Q * 4 + l
                    kis = [ki for ki in range(NB)
                           if kinds[qi][ki] != "skip" and ki in probs]
                    if not kis:
                        nc.vector.memset(attnT[:, h, l * P:(l + 1) * P], 0.0)
                    else:
                        pv = pv_ps.tile([P, HD + 1], F32, tag="pv")
                        for j, ki in enumerate(kis):
                            pt_t, pj = probs[ki]
                            nc.tensor.matmul(
                                pv[:], pt_t[:, pj, l * P:(l + 1) * P],
                                vaug_sb[:, ki, :],
                                start=(j == 0), stop=(j == len(kis) - 1))
                        recip = small.tile([P, 1], F32, tag="recip")
                        nc.vector.reciprocal(recip[:], pv[:, HD:HD + 1])
                        attn = small.tile([P, P], BF16, tag="attn")
                        nc.vector.tensor_scalar_mul(
                            out=attn[:], in0=pv[:, 0:HD], scalar1=recip[:])
                        tp = tp_ps.tile([P, P], BF16, tag="tp")
                        nc.tensor.transpose(tp[:], attn[:], ident[:])
                        nc.vector.tensor_copy(attnT[:, h, l * P:(l + 1) * P],
                                              tp[:])
                        if Q == NQC - 1 and h == HPG - 1:
                            # final chunk: its own o_proj interleaves into
                            # the last head's PV stream, one block behind
                            # so the attnT write has landed.
                            last_pv.append(l)
                            if len(last_pv) >= 2:
                                o_proj_block(Q, attnT, last_pv.pop(0),
                                             copy_eng=nc.scalar)

                present = [ki for ki in range(NB)
                           if any(kinds[Q * 4 + l][ki] != "skip"
                                  for l in range(4))]
                pairs = [present[i:i + 2] for i in range(0, len(present), 2)]
                npairs = len(pairs)
                for pi, pair in enumerate(pairs):
                    # union span over the pair so one rectangular activate
                    # covers both halves (the odd extra 128-col strip costs
                    # the PE far less than a second activate's overhead
                    # costs the scalar engine).
                    cols_by_ki = {
                        ki: [l for l in range(4)
                             if kinds[Q * 4 + l][ki] != "skip"]
                        for ki in pair}
                    lo = min(min(c) for c in cols_by_ki.values()) * P
                    hi = (max(max(c) for c in cols_by_ki.values()) + 1) * P
                    sc = qk_ps.tile([P, 2, QCH], F32, tag="sc")
                    for j, ki in enumerate(pair):
                        nc.tensor.matmul(
                            sc[:, j, lo:hi], krot_sb[:, ki * P:(ki + 1) * P],
                            qrot_sb[:, h, Q * QCH + lo:Q * QCH + hi],
                            start=True, stop=True)
                    pt = probs_pool.tile([P, 2, QCH], BF16, tag="pt")
                    nc.scalar.activation(
                        out=pt[:, 0:len(pair), lo:hi],
                        in_=sc[:, 0:len(pair), lo:hi],
                        func=mybir.ActivationFunctionType.Exp, scale=SCALE)
                    # multiplicative mask on the exp'd probs, on GPSIMD:
                    # keeps the mask off the PE (no extra matmul columns)
                    # and off vector (deep in PV-chain work here).  The
                    # masked slice is the diag ki == the LAST accumulation
                    # of its PV chain, so the extra hop's latency hides.
                    for j, ki in enumerate(pair):
                        for l in cols_by_ki[ki]:
                            kind = kinds[Q * 4 + l][ki]
                            if isinstance(kind, int):
                                nc.gpsimd.tensor_mul(
                                    pt[:, j, l * P:(l + 1) * P],
                                    pt[:, j, l * P:(l + 1) * P],
                                    mask_sb[:, kind, :])
                    for j, ki in enumerate(pair):
                        probs[ki] = (pt, j)
                    if npairs >= 4 and pi + 1 in (
                            (npairs + 3) // 4, 2 * ((npairs + 3) // 4),
                            3 * ((npairs + 3) // 4)):
                        fill(fill_state[0] + 1)
                    if Q == 0 and h == 0 and pi == 0 and rope_q:
                        emit_deferred_rope()
                    for ki in pair:
                        if ki >= Q * 4:
                            emit_pv(ki - Q * 4)
                fill(4)
            prev = (Q, attnT)
        for l in last_pv:
            o_proj_block(prev[0], prev[1], l, copy_eng=nc.scalar)

    nc.compile()
    return nc


_PROGRAM_CACHE = {}


def kernel(x, Wq, Wk, Wv, Wo, cos, sin, attention_mask):
    x = np.asarray(x, dtype=np.float32)
    Wq = np.asarray(Wq, dtype=np.float32)
    Wk = np.asarray(Wk, dtype=np.float32)
    Wv = np.asarray(Wv, dtype=np.float32)
    Wo = np.asarray(Wo, dtype=np.float32)
    cos = np.asarray(cos, dtype=np.float32)
    sin = np.asarray(sin, dtype=np.float32)
    mask = np.asarray(attention_mask, dtype=np.float32)[0, 0]

    kinds, blocks = _classify_mask(mask)
    key = (tuple(tuple(str(k) for k in row) for row in kinds), len(blocks))
    if key not in _PROGRAM_CACHE:
        _PROGRAM_CACHE[key] = _build_program(kinds, len(blocks))
    nc = _PROGRAM_CACHE[key]

    bf = ml_dtypes.bfloat16
    cosT = np.ascontiguousarray(cos[0, 0].T).astype(np.float32)
    sinT = np.ascontiguousarray(sin[0, 0].T).astype(np.float32)
    sinT[0:64] *= -1.0                                   # fold rotate_half sign
    sinP = np.concatenate([sinT[64:], sinT[:64]], axis=0)  # pre-rot 64
    cs = np.concatenate([cosT, sinP], axis=1).astype(bf)   # [HD, 2S]
    maskb = np.stack(blocks, axis=1) if blocks else None   # [P, nblk, P] bf16
    dd = np.arange(P)
    permM = (dd[:, None] == (dd[None, :] + 64) % P).astype(bf)

    in_maps = []
    for c in range(N_CORES):
        b, g = c // NKV, c % NKV
        d0, d1 = g * HPG * HD, (g + 1) * HPG * HD
        wkv = np.concatenate(
            [Wk[g * HD:(g + 1) * HD].T, Wv[g * HD:(g + 1) * HD].T], axis=1)
        m = {
            "xT": np.ascontiguousarray(x[b].T).astype(bf),
            "wqT": np.ascontiguousarray(Wq[d0:d1].T).astype(bf),
            "wkvT": np.ascontiguousarray(wkv).astype(bf),
            "woT": np.ascontiguousarray(Wo[:, d0:d1].T).astype(bf),
            "cs": cs,
            "perm": permM,
        }
        if maskb is not None:
            m["maskb"] = maskb
        in_maps.append(m)

    global _last_in_maps
    _last_in_maps = in_maps
    res = run_bass_kernel_spmd(nc, in_maps, list(range(N_CORES))).results
    out = np.zeros((B, S, H), np.float32)
    for c in range(N_CORES):
        out[c // NKV] += np.asarray(res[c]["out"]).astype(np.float32)
    return out
